# revision 28
# baseline (speedup 1.0000x reference)
"""Luong attention decoder step (B=64, S=128, H=1024, V=32000) on 8 trn2 cores.

Strategy (SPMD, one Bass program, per-core data differences only):
  - GRU gates sharded over the hidden dim (each core owns a 128-wide slice of
    h_new via a 384-col slice of W_ih/W_hh), AllGather #1 -> full h_newT.
  - Algebraic rewrite: q = h_new @ attn_W replaces the reference's
    enc_proj = enc @ attn_W.T (17 GFLOP -> 0.13 GFLOP); attn_b drops out
    exactly because softmax is invariant to per-row constants.
  - Attention (energies/softmax/context) sharded over batch (8 rows/core);
    energies = mult+reduce on DVE against q rows broadcast across partitions
    by K=64 selector matmuls.
  - context rows AllGather #2 (bf16), concat projection sharded over H (bf16),
    AllGather #3 (bf16), vocab projection sharded over V (bf16 weights,
    fp32 accumulation).
  - All biases ride an extra K=128 contraction chunk (row 0 = bias, rest 0)
    because K=1 matmuls crash the PE; no K<64 matmuls anywhere.
  - One batched DMA per weight tensor, issued on sync vs gpsimd queues so
    transfers overlap; GRU weights first (they head the dependency chain).
Outputs: logits [64,32000], h_new [1,64,1024], attn_weights [64,1,128].
"""

import os
import numpy as np
import ml_dtypes

import concourse.bass as bass
import concourse.bacc as bacc
import concourse.mybir as mybir
from concourse.tile import TileContext
from concourse.bass_utils import run_bass_kernel_spmd

B, S, H, V = 64, 128, 1024, 32000
NC = 8
BL = B // NC          # local batch rows per core
VL = V // NC          # vocab shard per core
HL = H // NC          # h_new slice per core
KC = H // 128         # contraction chunks of 128
f32 = mybir.dt.float32
bf16 = mybir.dt.bfloat16

_NC_CACHE = {}


def _build_nc(stage=40):
    nc = bacc.Bacc(num_devices=NC)

    def din(name, shape, dt):
        return nc.declare_dram_parameter(name, list(shape), dt, isOutput=False)

    # +128 rows on contraction dims: row H (resp. 2H) carries the bias,
    # matching activations carry a ones row there.
    xT = din("xT", (H + 128, B), f32)       # [emb[seq].T ; ones ; 0]  (repl)
    hT = din("hT", (H + 128, B), f32)       # [h.T ; ones ; 0]         (repl)
    h_sl = din("h_sl", (B, HL), f32)        # h[:, c*HL:(c+1)*HL]  (per-core)
    bsel = din("bsel", (B, BL * 128), f32)  # q-row broadcast sel  (per-core)
    wi3 = din("wi3", (H + 128, 3 * HL), f32)  # [W_ih.T slices ; b_ih ; 0]
    wh3 = din("wh3", (H + 128, 3 * HL), f32)
    attw = din("attw", (H, H), f32)         # attn_W natural       (replicated)
    enc = din("enc", (S, BL, H), f32)       # encoder slice        (per-core)
    catw = din("catw", (2 * H + 128, HL), bf16)  # [concat_W.T sl ; b ; 0]
    outw = din("outw", (H + 128, VL), bf16)      # [out_W.T sl ; out_b ; 0]
    ident = din("ident", (128, 128), f32)

    logits = nc.declare_dram_parameter("logits", [B, VL], f32, isOutput=True)
    hnT_out = nc.declare_dram_parameter("hnT", [H, B], f32, isOutput=True)
    attn_out = nc.declare_dram_parameter("attn", [BL, S], f32, isOutput=True)

    RG = [list(range(NC))]

    def rr(d, p=128):
        return d.rearrange("(n p) w -> p n w", p=p)

    with TileContext(nc) as tc:
        with (
            tc.tile_pool(name="persist", bufs=1) as pp,
            tc.tile_pool(name="wpool", bufs=1) as wp,
            tc.tile_pool(name="qrep", bufs=1) as qp,
            tc.tile_pool(name="one", bufs=1) as b1,
            tc.tile_pool(name="rot", bufs=2) as sp,
            tc.tile_pool(name="psum", bufs=2, space="PSUM") as ps,
            tc.tile_pool(name="dram", bufs=1, space="DRAM") as dp,
        ):
            def emit():
                # ---- GRU weights first on sync (head of the dep chain)
                wi3_sb = wp.tile([128, KC + 1, 3 * HL], f32, tag="wi3")
                nc.sync.dma_start(out=wi3_sb, in_=rr(wi3))
                wh3_sb = wp.tile([128, KC + 1, 3 * HL], f32, tag="wh3")
                nc.sync.dma_start(out=wh3_sb, in_=rr(wh3))
                xT_sb = wp.tile([128, KC + 1, B], f32, tag="xT")
                nc.sync.dma_start(out=xT_sb, in_=rr(xT))
                hT_sb = wp.tile([128, KC + 1, B], f32, tag="hT")
                nc.sync.dma_start(out=hT_sb, in_=rr(hT))
                h_sl_sb = pp.tile([B, HL], f32)
                nc.sync.dma_start(out=h_sl_sb, in_=h_sl[:, :])
                ident_sb = pp.tile([128, 128], f32)
                nc.scalar.dma_start(out=ident_sb, in_=ident[:, :])
                ident_bf = pp.tile([128, 128], bf16)
                nc.vector.tensor_copy(ident_bf, ident_sb)
                ones_ch = pp.tile([128, B], bf16)
                nc.vector.memset(ones_ch, 0.0)
                nc.vector.memset(ones_ch[0:1, :], 1.0)
                bsel_sb = pp.tile([B, BL, 128], f32)
                nc.scalar.dma_start(
                    out=bsel_sb,
                    in_=bsel.rearrange("b (j m) -> b j m", j=BL))

                # attw on sync after GRU weights; enc + outw-g0 on gpsimd
                attw_sb = wp.tile([128, KC, H], f32, tag="attw")
                nc.sync.dma_start(out=attw_sb, in_=rr(attw))
                enc_sb = wp.tile([S, BL, H], f32, tag="enc")
                nc.gpsimd.dma_start(out=enc_sb, in_=enc[:, :, :])
                outw_g0 = wp.tile([128, KC + 1, 2000], bf16, tag="outw")
                nc.gpsimd.dma_start(out=outw_g0, in_=rr(outw)[:, :, 0:2000])
                catw_sb = wp.tile([128, 2 * KC + 1, HL], bf16, tag="catw")
                nc.scalar.dma_start(out=catw_sb, in_=rr(catw))

                # ---- collective bounce buffers
                ag1_in = dp.tile([HL, B], f32)
                ag1_out = dp.tile([H, B], f32, addr_space="Shared")
                ag2_in = dp.tile([BL, H], bf16)
                ag2_out = dp.tile([B, H], bf16, addr_space="Shared")
                ag3_in = dp.tile([HL, B], bf16)
                ag3_out = dp.tile([H, B], bf16, addr_space="Shared")

                # ================= GRU (gate-slice sharded) =============
                if stage < 10:
                    return
                gi_ps = ps.tile([B, 3 * HL], f32, tag="pa")
                gh_ps = ps.tile([B, 3 * HL], f32, tag="pa")
                for k in range(KC + 1):
                    nc.tensor.matmul(gi_ps, xT_sb[:, k, :], wi3_sb[:, k, :],
                                     start=(k == 0), stop=(k == KC))
                for k in range(KC + 1):
                    nc.tensor.matmul(gh_ps, hT_sb[:, k, :], wh3_sb[:, k, :],
                                     start=(k == 0), stop=(k == KC))

                gh_sb = b1.tile([B, 3 * HL], f32, tag="ghs")
                nc.scalar.copy(gh_sb, gh_ps)
                rz_sum = b1.tile([B, 2 * HL], f32, tag="rz")
                nc.vector.tensor_add(rz_sum, gi_ps[:, 0:2 * HL],
                                     gh_sb[:, 0:2 * HL])
                rz = b1.tile([B, 2 * HL], f32, tag="rzs")
                nc.scalar.activation(rz, rz_sum,
                                     mybir.ActivationFunctionType.Sigmoid)
                n_in = b1.tile([B, HL], f32, tag="nin")
                nc.vector.tensor_mul(n_in, rz[:, 0:HL], gh_sb[:, 2 * HL:3 * HL])
                nc.vector.tensor_add(n_in, n_in, gi_ps[:, 2 * HL:3 * HL])
                n_t = b1.tile([B, HL], f32, tag="nt")
                nc.scalar.activation(n_t, n_in,
                                     mybir.ActivationFunctionType.Tanh)
                hmn = b1.tile([B, HL], f32, tag="hmn")
                nc.vector.tensor_tensor(hmn, h_sl_sb, n_t,
                                        mybir.AluOpType.subtract)
                nc.vector.tensor_mul(hmn, rz[:, HL:2 * HL], hmn)
                hn_c = b1.tile([B, HL], f32, tag="hnc")
                nc.vector.tensor_add(hn_c, n_t, hmn)

                hnT_ps = ps.tile([HL, B], f32, tag="pa")
                nc.tensor.transpose(hnT_ps, hn_c, ident_sb[:B, :B])
                hnT_sl = b1.tile([HL, B], f32, tag="hnT_sl")
                nc.scalar.copy(hnT_sl, hnT_ps)
                nc.gpsimd.dma_start(out=ag1_in[:, :], in_=hnT_sl)
                nc.gpsimd.collective_compute(
                    "AllGather", mybir.AluOpType.bypass, replica_groups=RG,
                    ins=[ag1_in[:, :]], outs=[ag1_out[:, :]],
                )
                nc.sync.dma_start(out=hnT_out[:, :], in_=ag1_out[:, :])
                hnT_sb = pp.tile([128, KC, B], f32)
                nc.sync.dma_start(out=hnT_sb, in_=rr(ag1_out))

                # ============= q = h_new @ attn_W (replicated) ==========
                if stage < 20:
                    return
                q_ps = ps.tile([B, H], f32, tag="pb")
                for k in range(KC):
                    for hf in range(2):
                        nc.tensor.matmul(
                            q_ps[:, hf * 512:(hf + 1) * 512],
                            hnT_sb[:, k, :],
                            attw_sb[:, k, hf * 512:(hf + 1) * 512],
                            start=(k == 0), stop=(k == KC - 1),
                        )
                q_sb = b1.tile([B, H], f32, tag="qsb")
                nc.vector.tensor_copy(q_sb, q_ps)

                # vocab-weight group 1 rides the freed attw slot
                outw_g1 = wp.tile([128, KC + 1, 2000], bf16, tag="attw")
                nc.sync.dma_start(out=outw_g1, in_=rr(outw)[:, :, 2000:4000])
                outw_g = [outw_g0, outw_g1]

                # ============ energies + softmax (batch sharded) ========
                # q_rep: q rows broadcast across partitions via K=64
                # selector matmuls, two local rows per round.
                if stage < 22:
                    return
                eT = pp.tile([S, BL], f32)
                for rnd in range(4):
                    q_rep = qp.tile([128, 2 * H], f32, tag="qrep")
                    for jj in range(2):
                        j = rnd * 2 + jj
                        for i in range(2):
                            rep_ps = ps.tile([128, 512], f32, tag="pr")
                            nc.tensor.matmul(
                                rep_ps, bsel_sb[:, j, :],
                                q_sb[:, i * 512:(i + 1) * 512],
                                start=True, stop=True,
                            )
                            nc.scalar.copy(
                                q_rep[:, jj * H + i * 512:
                                      jj * H + (i + 1) * 512],
                                rep_ps,
                            )
                    if stage < 25:
                        continue
                    for jj in range(2):
                        j = rnd * 2 + jj
                        prod = b1.tile([S, H], f32, tag="sc4")
                        nc.vector.tensor_mul(
                            prod, enc_sb[:, j, :],
                            q_rep[:, jj * H:(jj + 1) * H])
                        nc.vector.reduce_sum(
                            out=eT[:, j:j + 1], in_=prod,
                            axis=mybir.AxisListType.X)

                if stage < 25:
                    return
                e_ps = ps.tile([BL, S], f32, tag="pa")
                nc.tensor.transpose(e_ps, eT, ident_sb)
                mx = pp.tile([BL, 1], f32, tag="mx")
                nc.vector.reduce_max(out=mx, in_=e_ps,
                                     axis=mybir.AxisListType.X)
                negm = pp.tile([BL, 1], f32, tag="negm")
                nc.vector.tensor_scalar_mul(negm, mx, -1.0)
                p_sb = pp.tile([BL, S], f32, tag="p")
                ssum = pp.tile([BL, 1], f32, tag="ssum")
                nc.scalar.activation(
                    p_sb, e_ps, mybir.ActivationFunctionType.Exp,
                    bias=negm, scale=1.0, accum_out=ssum,
                )
                rsum = pp.tile([BL, 1], f32, tag="rsum")
                nc.vector.reciprocal(rsum, ssum)
                # attn rows live in a 64-partition tile so the transpose
                # runs at K=64 (rows BL..63 are zero).
                attn_sb = pp.tile([B, S], f32, tag="attn")
                nc.vector.memset(attn_sb, 0.0)
                nc.vector.tensor_scalar_mul(attn_sb[:BL, :], p_sb, rsum)
                nc.sync.dma_start(out=attn_out[:, :], in_=attn_sb[:BL, :])
                aT_ps = ps.tile([S, B], f32, tag="pa")
                nc.tensor.transpose(aT_ps, attn_sb, ident_sb[:B, :B])
                aT_sb = pp.tile([S, BL], f32, tag="aTs")
                nc.scalar.copy(aT_sb, aT_ps[:, :BL])

                if stage < 30:
                    return
                # ============= context rows (batch sharded) =============
                for j in range(BL):
                    ctx_ps = ps.tile([1, H], f32, tag="pb")
                    for hf in range(2):
                        nc.tensor.matmul(
                            ctx_ps[:, hf * 512:(hf + 1) * 512],
                            aT_sb[:, j:j + 1],
                            enc_sb[:, j, hf * 512:(hf + 1) * 512],
                            start=True, stop=True,
                        )
                    ctx_row = sp.tile([1, H], bf16, tag="qrow")
                    nc.any.tensor_copy(ctx_row, ctx_ps)
                    nc.gpsimd.dma_start(out=ag2_in[j:j + 1, :], in_=ctx_row)
                nc.gpsimd.collective_compute(
                    "AllGather", mybir.AluOpType.bypass, replica_groups=RG,
                    ins=[ag2_in[:, :]], outs=[ag2_out[:, :]],
                )

                # concat input lhsT: [h_newT ; contextT ; ones row] in bf16
                catin = []
                for k in range(KC):
                    t = pp.tile([128, B], bf16, tag=f"ci{k}")
                    nc.scalar.copy(t, hnT_sb[:, k, :])
                    catin.append(t)
                ctxf = b1.tile([B, KC, 128], bf16, tag="ctxf")
                nc.sync.dma_start(
                    out=ctxf, in_=ag2_out.rearrange("b (k p) -> b k p", p=128))
                for k in range(KC):
                    cT_ps = ps.tile([128, B], bf16, tag="pa")
                    nc.tensor.transpose(
                        cT_ps, ctxf[:, k, :], ident_bf[:B, :B])
                    t = pp.tile([128, B], bf16, tag=f"ci{KC + k}")
                    nc.vector.tensor_copy(t, cT_ps)
                    catin.append(t)
                catin.append(ones_ch)

                # ============ concat projection (H sharded) =============
                co_ps = ps.tile([HL, B], f32, tag="pa")
                for k in range(2 * KC + 1):
                    nc.tensor.matmul(co_ps, catw_sb[:, k, :], catin[k],
                                     start=(k == 0), stop=(k == 2 * KC))
                co_sb = b1.tile([HL, B], bf16, tag="cos")
                nc.scalar.activation(co_sb, co_ps,
                                     mybir.ActivationFunctionType.Tanh)
                nc.gpsimd.dma_start(out=ag3_in[:, :], in_=co_sb)
                nc.gpsimd.collective_compute(
                    "AllGather", mybir.AluOpType.bypass, replica_groups=RG,
                    ins=[ag3_in[:, :]], outs=[ag3_out[:, :]],
                )
                coT_sb = pp.tile([128, KC, B], bf16)
                nc.sync.dma_start(out=coT_sb, in_=rr(ag3_out))

                if stage < 40:
                    return
                # ============ vocab projection (V sharded) ==============
                for nt in range(8):
                    g, hf = nt // 4, nt % 4
                    lo_ps = ps.tile([B, 500], f32, tag="pb")
                    for k in range(KC + 1):
                        lhsT = (coT_sb[:, k, :] if k < KC else ones_ch)
                        nc.tensor.matmul(
                            lo_ps, lhsT,
                            outw_g[g][:, k, hf * 500:(hf + 1) * 500],
                            start=(k == 0), stop=(k == KC),
                        )
                    lo_sb = sp.tile([B, 500], f32, tag="los")
                    nc.vector.tensor_copy(lo_sb, lo_ps)
                    nc.sync.dma_start(
                        out=logits[:, nt * 500:(nt + 1) * 500], in_=lo_sb)

            emit()
    nc.finalize()
    return nc


def _get_nc():
    if "nc" not in _NC_CACHE:
        _NC_CACHE["nc"] = _build_nc(int(os.environ.get("KSTAGE", "40")))
    return _NC_CACHE["nc"]


def kernel(input_seq, last_hidden, encoder_outputs, emb, W_ih, W_hh, b_ih, b_hh,
           attn_W, attn_b, concat_W, concat_b, out_W, out_b):
    f = np.float32
    bf = ml_dtypes.bfloat16
    seq = np.asarray(input_seq).astype(np.int64)
    x = np.ascontiguousarray(np.asarray(emb, f)[seq])            # [B, H]
    h = np.ascontiguousarray(np.asarray(last_hidden, f)[0])      # [B, H]

    def ext_act(m):
        # [H, B] activation extended with a ones row + 127 zero rows
        out = np.zeros((H + 128, B), f)
        out[:H] = m
        out[H] = 1.0
        return out

    xT = ext_act(x.T)
    hT = ext_act(h.T)
    WiT = np.asarray(W_ih, f).T                                  # [H, 3H]
    WhT = np.asarray(W_hh, f).T
    bi = np.asarray(b_ih, f)
    bh = np.asarray(b_hh, f)
    attw = np.ascontiguousarray(np.asarray(attn_W, f))           # natural [H, H]
    encf = np.ascontiguousarray(np.asarray(encoder_outputs, f))  # [S, B, H]
    catWT = np.asarray(concat_W, f).T                            # [2H, H]
    catb = np.asarray(concat_b, f)
    outWT = np.asarray(out_W, f).T                               # [H, V]
    outb = np.asarray(out_b, f)
    ident = np.eye(128, dtype=f)

    in_maps = []
    for c in range(NC):
        cols = lambda m: np.concatenate(
            [m[:, g * H + c * HL:g * H + (c + 1) * HL] for g in range(3)],
            axis=1)
        gsl = np.zeros((H + 128, 3 * HL), f)
        gsl[:H] = cols(WiT)
        gsl[H] = np.concatenate(
            [bi[g * H + c * HL:g * H + (c + 1) * HL] for g in range(3)])
        gsh = np.zeros((H + 128, 3 * HL), f)
        gsh[:H] = cols(WhT)
        gsh[H] = np.concatenate(
            [bh[g * H + c * HL:g * H + (c + 1) * HL] for g in range(3)])
        bselm = np.zeros((B, BL, 128), f)
        for j in range(BL):
            bselm[c * BL + j, j, :] = 1.0
        catw_c = np.zeros((2 * H + 128, HL), f)
        catw_c[:2 * H] = catWT[:, c * HL:(c + 1) * HL]
        catw_c[2 * H] = catb[c * HL:(c + 1) * HL]
        outw_c = np.zeros((H + 128, VL), f)
        outw_c[:H] = outWT[:, c * VL:(c + 1) * VL]
        outw_c[H] = outb[c * VL:(c + 1) * VL]
        in_maps.append({
            "xT": xT, "hT": hT,
            "h_sl": np.ascontiguousarray(h[:, c * HL:(c + 1) * HL]),
            "bsel": bselm.reshape(B, BL * 128),
            "wi3": gsl, "wh3": gsh,
            "attw": attw,
            "enc": np.ascontiguousarray(encf[:, c * BL:(c + 1) * BL, :]),
            "catw": catw_c.astype(bf),
            "outw": outw_c.astype(bf),
            "ident": ident,
        })

    nc = _get_nc()
    results = run_bass_kernel_spmd(nc, in_maps, list(range(NC))).results

    logits = np.concatenate([results[c]["logits"] for c in range(NC)], axis=1)
    h_new = np.ascontiguousarray(results[0]["hnT"].T)[None]
    attn = np.concatenate([results[c]["attn"] for c in range(NC)], axis=0)
    return (logits.astype(f), h_new.astype(f), attn[:, None, :].astype(f))


# revision 29
# speedup vs baseline: 1.0529x; 1.0529x over previous
"""Luong attention decoder step (B=64, S=128, H=1024, V=32000) on 8 trn2 cores.

Strategy (SPMD, one Bass program, per-core data differences only):
  - GRU gates sharded over the hidden dim (each core owns a 128-wide slice of
    h_new via a 384-col slice of W_ih/W_hh), AllGather #1 -> full h_newT.
  - Algebraic rewrite: q = h_new @ attn_W replaces the reference's
    enc_proj = enc @ attn_W.T (17 GFLOP -> 0.13 GFLOP); attn_b drops out
    exactly because softmax is invariant to per-row constants.
  - Attention (energies/softmax/context) sharded over batch (8 rows/core);
    energies = mult+reduce on DVE against q rows broadcast across partitions
    by K=64 selector matmuls.
  - context rows AllGather #2 (bf16), concat projection sharded over H (bf16),
    AllGather #3 (bf16), vocab projection sharded over V (bf16 weights,
    fp32 accumulation).
  - All biases ride an extra K=128 contraction chunk (row 0 = bias, rest 0)
    because K=1 matmuls crash the PE; no K<64 matmuls anywhere.
  - One batched DMA per weight tensor, issued on sync vs gpsimd queues so
    transfers overlap; GRU weights first (they head the dependency chain).
Outputs: logits [64,32000], h_new [1,64,1024], attn_weights [64,1,128].
"""

import os
import numpy as np
import ml_dtypes

import concourse.bass as bass
import concourse.bacc as bacc
import concourse.mybir as mybir
from concourse.tile import TileContext
from concourse.bass_utils import run_bass_kernel_spmd

B, S, H, V = 64, 128, 1024, 32000
NC = 8
BL = B // NC          # local batch rows per core
VL = V // NC          # vocab shard per core
HL = H // NC          # h_new slice per core
KC = H // 128         # contraction chunks of 128
f32 = mybir.dt.float32
bf16 = mybir.dt.bfloat16

_NC_CACHE = {}


def _build_nc(stage=40):
    nc = bacc.Bacc(num_devices=NC)

    def din(name, shape, dt):
        return nc.declare_dram_parameter(name, list(shape), dt, isOutput=False)

    # +128 rows on contraction dims: row H (resp. 2H) carries the bias,
    # matching activations carry a ones row there.
    xT = din("xT", (H + 128, B), f32)       # [emb[seq].T ; ones ; 0]  (repl)
    hT = din("hT", (H + 128, B), f32)       # [h.T ; ones ; 0]         (repl)
    h_sl = din("h_sl", (B, HL), f32)        # h[:, c*HL:(c+1)*HL]  (per-core)
    bsel = din("bsel", (B, BL * 128), f32)  # q-row broadcast sel  (per-core)
    wi3 = din("wi3", (H + 128, 3 * HL), f32)  # [W_ih.T slices ; b_ih ; 0]
    wh3 = din("wh3", (H + 128, 3 * HL), f32)
    attw = din("attw", (H, H), f32)         # attn_W natural       (replicated)
    enc = din("enc", (S, BL, H), f32)       # encoder slice        (per-core)
    catw = din("catw", (2 * H + 128, HL), bf16)  # [concat_W.T sl ; b ; 0]
    outw = din("outw", (H + 128, VL), bf16)      # [out_W.T sl ; out_b ; 0]
    ident = din("ident", (128, 128), f32)

    logits = nc.declare_dram_parameter("logits", [B, VL], f32, isOutput=True)
    hnT_out = nc.declare_dram_parameter("hnT", [H, B], f32, isOutput=True)
    attn_out = nc.declare_dram_parameter("attn", [BL, S], f32, isOutput=True)

    RG = [list(range(NC))]

    def rr(d, p=128):
        return d.rearrange("(n p) w -> p n w", p=p)

    with TileContext(nc) as tc:
        with (
            tc.tile_pool(name="persist", bufs=1) as pp,
            tc.tile_pool(name="wpool", bufs=1) as wp,
            tc.tile_pool(name="qrep", bufs=1) as qp,
            tc.tile_pool(name="one", bufs=1) as b1,
            tc.tile_pool(name="rot", bufs=2) as sp,
            tc.tile_pool(name="psum", bufs=2, space="PSUM") as ps,
            tc.tile_pool(name="dram", bufs=1, space="DRAM") as dp,
        ):
            def emit():
                # ---- GRU weights first on sync (head of the dep chain)
                xT_sb = wp.tile([128, KC + 1, B], f32, tag="xT")
                nc.sync.dma_start(out=xT_sb, in_=rr(xT))
                hT_sb = wp.tile([128, KC + 1, B], f32, tag="hT")
                nc.sync.dma_start(out=hT_sb, in_=rr(hT))
                wi3_sb = wp.tile([128, KC + 1, 3 * HL], f32, tag="wi3")
                nc.sync.dma_start(out=wi3_sb, in_=rr(wi3))
                h_sl_sb = pp.tile([B, HL], f32)
                nc.sync.dma_start(out=h_sl_sb, in_=h_sl[:, :])
                wh3_sb = wp.tile([128, KC + 1, 3 * HL], f32, tag="wh3")
                nc.scalar.dma_start(out=wh3_sb, in_=rr(wh3))
                ident_sb = pp.tile([128, 128], f32)
                nc.scalar.dma_start(out=ident_sb, in_=ident[:, :])
                ident_bf = pp.tile([128, 128], bf16)
                nc.vector.tensor_copy(ident_bf, ident_sb)
                ones_ch = pp.tile([128, B], bf16)
                nc.vector.memset(ones_ch, 0.0)
                nc.vector.memset(ones_ch[0:1, :], 1.0)
                bsel_sb = pp.tile([B, BL, 128], f32)
                nc.scalar.dma_start(
                    out=bsel_sb,
                    in_=bsel.rearrange("b (j m) -> b j m", j=BL))
                catw_sb = wp.tile([128, 2 * KC + 1, HL], bf16, tag="catw")
                nc.scalar.dma_start(out=catw_sb, in_=rr(catw))
                # preload the Exp activation table off the critical path
                exp_warm = pp.tile([1, 1], f32)
                nc.vector.memset(exp_warm, 0.0)
                nc.scalar.activation(exp_warm, exp_warm,
                                     mybir.ActivationFunctionType.Exp)

                attw_sb = wp.tile([128, KC, H], f32, tag="attw")
                nc.sync.dma_start(out=attw_sb, in_=rr(attw))
                enc_sb = wp.tile([S, BL, H], f32, tag="enc")
                nc.sync.dma_start(out=enc_sb, in_=enc[:, :, :])
                outw_g0 = wp.tile([128, KC + 1, 2000], bf16, tag="outw")
                nc.sync.dma_start(out=outw_g0, in_=rr(outw)[:, :, 0:2000])

                # ---- collective bounce buffers
                ag1_in = dp.tile([HL, B], f32)
                ag1_out = dp.tile([H, B], f32, addr_space="Shared")
                ag2_in = dp.tile([BL, H], bf16)
                ag2_out = dp.tile([B, H], bf16, addr_space="Shared")
                ag3_in = dp.tile([HL, B], bf16)
                ag3_out = dp.tile([H, B], bf16, addr_space="Shared")

                # ================= GRU (gate-slice sharded) =============
                if stage < 10:
                    return
                gi_ps = ps.tile([B, 3 * HL], f32, tag="pa")
                gh_ps = ps.tile([B, 3 * HL], f32, tag="pa")
                for k in range(KC + 1):
                    nc.tensor.matmul(gi_ps, xT_sb[:, k, :], wi3_sb[:, k, :],
                                     start=(k == 0), stop=(k == KC))
                for k in range(KC + 1):
                    nc.tensor.matmul(gh_ps, hT_sb[:, k, :], wh3_sb[:, k, :],
                                     start=(k == 0), stop=(k == KC))

                gh_sb = b1.tile([B, 3 * HL], f32, tag="ghs")
                nc.scalar.copy(gh_sb, gh_ps)
                rz_sum = b1.tile([B, 2 * HL], f32, tag="rz")
                nc.vector.tensor_add(rz_sum, gi_ps[:, 0:2 * HL],
                                     gh_sb[:, 0:2 * HL])
                rz = b1.tile([B, 2 * HL], f32, tag="rzs")
                nc.scalar.activation(rz, rz_sum,
                                     mybir.ActivationFunctionType.Sigmoid)
                n_in = b1.tile([B, HL], f32, tag="nin")
                nc.vector.tensor_mul(n_in, rz[:, 0:HL], gh_sb[:, 2 * HL:3 * HL])
                nc.vector.tensor_add(n_in, n_in, gi_ps[:, 2 * HL:3 * HL])
                n_t = b1.tile([B, HL], f32, tag="nt")
                nc.scalar.activation(n_t, n_in,
                                     mybir.ActivationFunctionType.Tanh)
                hmn = b1.tile([B, HL], f32, tag="hmn")
                nc.vector.tensor_tensor(hmn, h_sl_sb, n_t,
                                        mybir.AluOpType.subtract)
                nc.vector.tensor_mul(hmn, rz[:, HL:2 * HL], hmn)
                hn_c = b1.tile([B, HL], f32, tag="hnc")
                nc.vector.tensor_add(hn_c, n_t, hmn)

                hnT_ps = ps.tile([HL, B], f32, tag="pa")
                nc.tensor.transpose(hnT_ps, hn_c, ident_sb[:B, :B])
                hnT_sl = b1.tile([HL, B], f32, tag="hnT_sl")
                nc.scalar.copy(hnT_sl, hnT_ps)
                nc.gpsimd.dma_start(out=ag1_in[:, :], in_=hnT_sl)
                nc.gpsimd.collective_compute(
                    "AllGather", mybir.AluOpType.bypass, replica_groups=RG,
                    ins=[ag1_in[:, :]], outs=[ag1_out[:, :]],
                )
                nc.gpsimd.dma_start(out=hnT_out[:, :], in_=ag1_out[:, :])
                hnT_sb = pp.tile([128, KC, B], f32)
                nc.sync.dma_start(out=hnT_sb, in_=rr(ag1_out))
                # concat lhsT h_newT chunks in bf16 (ready long before AG2)
                catin = []
                for k in range(KC):
                    t = pp.tile([128, B], bf16, tag=f"ci{k}")
                    nc.scalar.copy(t, hnT_sb[:, k, :])
                    catin.append(t)

                # ============= q = h_new @ attn_W (replicated) ==========
                if stage < 20:
                    return
                q_ps = ps.tile([B, H], f32, tag="pb")
                for k in range(KC):
                    for hf in range(2):
                        nc.tensor.matmul(
                            q_ps[:, hf * 512:(hf + 1) * 512],
                            hnT_sb[:, k, :],
                            attw_sb[:, k, hf * 512:(hf + 1) * 512],
                            start=(k == 0), stop=(k == KC - 1),
                        )
                q_sb = b1.tile([B, H], f32, tag="qsb")
                nc.vector.tensor_copy(q_sb, q_ps)

                # vocab-weight group 1 rides the freed attw slot
                outw_g1 = wp.tile([128, KC + 1, 2000], bf16, tag="attw")
                nc.sync.dma_start(out=outw_g1, in_=rr(outw)[:, :, 2000:4000])
                outw_g = [outw_g0, outw_g1]

                # ============ energies + softmax (batch sharded) ========
                # q_rep: q rows broadcast across partitions via K=64
                # selector matmuls, two local rows per round.
                if stage < 22:
                    return
                eT = pp.tile([S, BL], f32)
                for rnd in range(4):
                    q_rep = qp.tile([128, 2 * H], f32, tag="qrep")
                    for jj in range(2):
                        j = rnd * 2 + jj
                        for i in range(2):
                            rep_ps = ps.tile([128, 512], f32, tag="pr")
                            nc.tensor.matmul(
                                rep_ps, bsel_sb[:, j, :],
                                q_sb[:, i * 512:(i + 1) * 512],
                                start=True, stop=True,
                            )
                            nc.scalar.copy(
                                q_rep[:, jj * H + i * 512:
                                      jj * H + (i + 1) * 512],
                                rep_ps,
                            )
                    if stage < 25:
                        continue
                    for jj in range(2):
                        j = rnd * 2 + jj
                        prod = b1.tile([S, H], f32, tag="sc4")
                        nc.vector.tensor_mul(
                            prod, enc_sb[:, j, :],
                            q_rep[:, jj * H:(jj + 1) * H])
                        nc.vector.reduce_sum(
                            out=eT[:, j:j + 1], in_=prod,
                            axis=mybir.AxisListType.X)

                if stage < 25:
                    return
                e_ps = ps.tile([BL, S], f32, tag="pa")
                nc.tensor.transpose(e_ps, eT, ident_sb)
                mx = pp.tile([BL, 1], f32, tag="mx")
                nc.vector.reduce_max(out=mx, in_=e_ps,
                                     axis=mybir.AxisListType.X)
                negm = pp.tile([BL, 1], f32, tag="negm")
                nc.vector.tensor_scalar_mul(negm, mx, -1.0)
                p_sb = pp.tile([BL, S], f32, tag="p")
                ssum = pp.tile([BL, 1], f32, tag="ssum")
                nc.scalar.activation(
                    p_sb, e_ps, mybir.ActivationFunctionType.Exp,
                    bias=negm, scale=1.0, accum_out=ssum,
                )
                rsum = pp.tile([BL, 1], f32, tag="rsum")
                nc.vector.reciprocal(rsum, ssum)
                # attn rows live in a 64-partition tile so the transpose
                # runs at K=64 (rows BL..63 are zero).
                attn_sb = pp.tile([B, S], f32, tag="attn")
                nc.vector.memset(attn_sb, 0.0)
                nc.vector.tensor_scalar_mul(attn_sb[:BL, :], p_sb, rsum)
                nc.sync.dma_start(out=attn_out[:, :], in_=attn_sb[:BL, :])
                aT_ps = ps.tile([S, B], f32, tag="pa")
                nc.tensor.transpose(aT_ps, attn_sb, ident_sb[:B, :B])
                aT_sb = pp.tile([S, BL], f32, tag="aTs")
                nc.scalar.copy(aT_sb, aT_ps[:, :BL])

                if stage < 30:
                    return
                # ============= context rows (batch sharded) =============
                for j in range(BL):
                    ctx_ps = ps.tile([1, H], f32, tag="pb")
                    for hf in range(2):
                        nc.tensor.matmul(
                            ctx_ps[:, hf * 512:(hf + 1) * 512],
                            aT_sb[:, j:j + 1],
                            enc_sb[:, j, hf * 512:(hf + 1) * 512],
                            start=True, stop=True,
                        )
                    ctx_row = sp.tile([1, H], bf16, tag="qrow")
                    eng = nc.vector if j % 2 == 0 else nc.scalar
                    if j % 2 == 0:
                        nc.vector.tensor_copy(ctx_row, ctx_ps)
                    else:
                        nc.scalar.copy(ctx_row, ctx_ps)
                    nc.gpsimd.dma_start(out=ag2_in[j:j + 1, :], in_=ctx_row)
                nc.gpsimd.collective_compute(
                    "AllGather", mybir.AluOpType.bypass, replica_groups=RG,
                    ins=[ag2_in[:, :]], outs=[ag2_out[:, :]],
                )

                # contextT chunks appended to catin
                ctxf = b1.tile([B, KC, 128], bf16, tag="ctxf")
                nc.sync.dma_start(
                    out=ctxf, in_=ag2_out.rearrange("b (k p) -> b k p", p=128))
                for k in range(KC):
                    cT_ps = ps.tile([128, B], bf16, tag="pa")
                    nc.tensor.transpose(
                        cT_ps, ctxf[:, k, :], ident_bf[:B, :B])
                    t = pp.tile([128, B], bf16, tag=f"ci{KC + k}")
                    nc.vector.tensor_copy(t, cT_ps)
                    catin.append(t)
                catin.append(ones_ch)

                # ============ concat projection (H sharded) =============
                co_ps = ps.tile([HL, B], f32, tag="pa")
                for k in range(2 * KC + 1):
                    nc.tensor.matmul(co_ps, catw_sb[:, k, :], catin[k],
                                     start=(k == 0), stop=(k == 2 * KC))
                co_sb = b1.tile([HL, B], bf16, tag="cos")
                nc.scalar.activation(co_sb, co_ps,
                                     mybir.ActivationFunctionType.Tanh)
                nc.gpsimd.dma_start(out=ag3_in[:, :], in_=co_sb)
                nc.gpsimd.collective_compute(
                    "AllGather", mybir.AluOpType.bypass, replica_groups=RG,
                    ins=[ag3_in[:, :]], outs=[ag3_out[:, :]],
                )
                coT_sb = pp.tile([128, KC, B], bf16)
                nc.sync.dma_start(out=coT_sb, in_=rr(ag3_out))

                if stage < 40:
                    return
                # ============ vocab projection (V sharded) ==============
                for nt in range(8):
                    g, hf = nt // 4, nt % 4
                    lo_ps = ps.tile([B, 500], f32, tag="pb")
                    for k in range(KC + 1):
                        lhsT = (coT_sb[:, k, :] if k < KC else ones_ch)
                        nc.tensor.matmul(
                            lo_ps, lhsT,
                            outw_g[g][:, k, hf * 500:(hf + 1) * 500],
                            start=(k == 0), stop=(k == KC),
                        )
                    lo_sb = sp.tile([B, 500], f32, tag="los")
                    nc.vector.tensor_copy(lo_sb, lo_ps)
                    nc.sync.dma_start(
                        out=logits[:, nt * 500:(nt + 1) * 500], in_=lo_sb)

            emit()
    nc.finalize()
    return nc


def _get_nc():
    if "nc" not in _NC_CACHE:
        _NC_CACHE["nc"] = _build_nc(int(os.environ.get("KSTAGE", "40")))
    return _NC_CACHE["nc"]


def kernel(input_seq, last_hidden, encoder_outputs, emb, W_ih, W_hh, b_ih, b_hh,
           attn_W, attn_b, concat_W, concat_b, out_W, out_b):
    f = np.float32
    bf = ml_dtypes.bfloat16
    seq = np.asarray(input_seq).astype(np.int64)
    x = np.ascontiguousarray(np.asarray(emb, f)[seq])            # [B, H]
    h = np.ascontiguousarray(np.asarray(last_hidden, f)[0])      # [B, H]

    def ext_act(m):
        # [H, B] activation extended with a ones row + 127 zero rows
        out = np.zeros((H + 128, B), f)
        out[:H] = m
        out[H] = 1.0
        return out

    xT = ext_act(x.T)
    hT = ext_act(h.T)
    WiT = np.asarray(W_ih, f).T                                  # [H, 3H]
    WhT = np.asarray(W_hh, f).T
    bi = np.asarray(b_ih, f)
    bh = np.asarray(b_hh, f)
    attw = np.ascontiguousarray(np.asarray(attn_W, f))           # natural [H, H]
    encf = np.ascontiguousarray(np.asarray(encoder_outputs, f))  # [S, B, H]
    catWT = np.asarray(concat_W, f).T                            # [2H, H]
    catb = np.asarray(concat_b, f)
    outWT = np.asarray(out_W, f).T                               # [H, V]
    outb = np.asarray(out_b, f)
    ident = np.eye(128, dtype=f)

    in_maps = []
    for c in range(NC):
        cols = lambda m: np.concatenate(
            [m[:, g * H + c * HL:g * H + (c + 1) * HL] for g in range(3)],
            axis=1)
        gsl = np.zeros((H + 128, 3 * HL), f)
        gsl[:H] = cols(WiT)
        gsl[H] = np.concatenate(
            [bi[g * H + c * HL:g * H + (c + 1) * HL] for g in range(3)])
        gsh = np.zeros((H + 128, 3 * HL), f)
        gsh[:H] = cols(WhT)
        gsh[H] = np.concatenate(
            [bh[g * H + c * HL:g * H + (c + 1) * HL] for g in range(3)])
        bselm = np.zeros((B, BL, 128), f)
        for j in range(BL):
            bselm[c * BL + j, j, :] = 1.0
        catw_c = np.zeros((2 * H + 128, HL), f)
        catw_c[:2 * H] = catWT[:, c * HL:(c + 1) * HL]
        catw_c[2 * H] = catb[c * HL:(c + 1) * HL]
        outw_c = np.zeros((H + 128, VL), f)
        outw_c[:H] = outWT[:, c * VL:(c + 1) * VL]
        outw_c[H] = outb[c * VL:(c + 1) * VL]
        in_maps.append({
            "xT": xT, "hT": hT,
            "h_sl": np.ascontiguousarray(h[:, c * HL:(c + 1) * HL]),
            "bsel": bselm.reshape(B, BL * 128),
            "wi3": gsl, "wh3": gsh,
            "attw": attw,
            "enc": np.ascontiguousarray(encf[:, c * BL:(c + 1) * BL, :]),
            "catw": catw_c.astype(bf),
            "outw": outw_c.astype(bf),
            "ident": ident,
        })

    nc = _get_nc()
    results = run_bass_kernel_spmd(nc, in_maps, list(range(NC))).results

    logits = np.concatenate([results[c]["logits"] for c in range(NC)], axis=1)
    h_new = np.ascontiguousarray(results[0]["hnT"].T)[None]
    attn = np.concatenate([results[c]["attn"] for c in range(NC)], axis=0)
    return (logits.astype(f), h_new.astype(f), attn[:, None, :].astype(f))
